# revision 29
# baseline (speedup 1.0000x reference)
"""Windowed (4,4,4) multi-head attention block for Trainium2, 8-core data parallel.

Full computation: x -> window partition -> QKV -> RMSNorm(q,k) -> per-window
softmax attention -> out proj + bias -> window unpartition.

Sharding: 256 windows * 64 tokens = 16384 tokens split as 2048 tokens (32
windows) per core, pure data parallel. Host does the (free) window
partition / transposes; device does GEMMs + attention in bf16 with fp32
accumulation.
"""

import os
import sys

import numpy as np

sys.path.insert(0, "/opt/trn_rl_repo")

import ml_dtypes

BF16 = ml_dtypes.bfloat16

DIM = 1152
HEADS = 16
HD = 72
NCORES = 8
TOK_PER_CORE = 2048
NTILES = TOK_PER_CORE // 128  # 16 token tiles of 128 (2 windows each)
KT = DIM // 128  # 9 contraction k-tiles
OC3 = 3 * DIM  # 3456
HDP = 80  # q/k head stride padded to a multiple of 16 (xbar out-partition req)
QKP = HEADS * HDP  # 1280
OCP = 2 * QKP + DIM  # 3712 padded qkv channels
SCALE = HD ** -0.5
EPS = 1e-6

_CACHE = {}


def _build_nc(use_wqk):
    import concourse.bass as bass
    import concourse.mybir as mybir
    import concourse.tile as tile
    from concourse import bacc
    from concourse.masks import make_identity

    f32 = mybir.dt.float32
    bf16 = mybir.dt.bfloat16
    ALU = mybir.AluOpType
    ACTF = mybir.ActivationFunctionType

    nc = bacc.Bacc("TRN2", target_bir_lowering=False, debug=False)

    xt_d = nc.dram_tensor("xt", [KT, 128, TOK_PER_CORE], bf16, kind="ExternalInput")
    wqkv_d = nc.dram_tensor("wqkv", [KT, 128, OC3], bf16, kind="ExternalInput")
    wproj_d = nc.dram_tensor("wproj", [KT, 128, DIM], bf16, kind="ExternalInput")
    bproj_d = nc.dram_tensor("bproj", [KT, 128], f32, kind="ExternalInput")
    if use_wqk:
        wqk_d = nc.dram_tensor("wqk", [1, DIM], bf16, kind="ExternalInput")
    out_d = nc.dram_tensor("out", [KT, 128, TOK_PER_CORE], f32, kind="ExternalOutput")

    with tile.TileContext(nc) as tc:
        from contextlib import ExitStack

        with ExitStack() as ctx:
            const = ctx.enter_context(tc.tile_pool(name="const", bufs=1))
            ident = const.tile([128, 128], bf16)
            make_identity(nc, ident)
            # additive mask: 0 on the two diagonal 64x64 windows, -1e9 across
            mask = const.tile([128, 128], f32)
            nc.gpsimd.memset(mask, 0.0)
            nc.gpsimd.memset(mask[0:64, 64:128], -1e9)
            nc.gpsimd.memset(mask[64:128, 0:64], -1e9)
            eps_sb = const.tile([128, 1], f32)
            nc.vector.memset(eps_sb, EPS)
            zero_sb = const.tile([128, 1], f32)
            nc.vector.memset(zero_sb, 0.0)

            wq_sb = const.tile([128, KT, OC3], bf16)
            nc.sync.dma_start(out=wq_sb, in_=wqkv_d[:, :, :].rearrange("k p c -> p k c"))
            wp_sb = const.tile([128, KT, DIM], bf16)
            nc.sync.dma_start(out=wp_sb, in_=wproj_d[:, :, :].rearrange("k p c -> p k c"))
            bp_sb = const.tile([128, KT], f32)
            nc.sync.dma_start(out=bp_sb, in_=bproj_d[:, :].rearrange("k p -> p k"))
            if use_wqk:
                wqk_sb = const.tile([128, DIM], bf16)
                nc.gpsimd.dma_start(
                    out=wqk_sb,
                    in_=bass.AP(wqk_d.ap().tensor, 0, [[0, 128], [1, DIM]]),
                )

            o_all = const.tile([128, NTILES, DIM], bf16)

            # ---------------- phase 1+2: QKV GEMM + attention, per 128-token tile
            with (
                tc.tile_pool(name="xin", bufs=3) as xpool,
                tc.tile_pool(name="qkv", bufs=2) as qkvpool,
                tc.tile_pool(name="sq", bufs=2) as sqpool,
                tc.tile_pool(name="stats", bufs=2) as stats,
                tc.tile_pool(name="qkts", bufs=3) as qktsb,
                tc.tile_pool(name="psb", bufs=3) as ppool,
                tc.tile_pool(name="ptsb", bufs=3) as ptsb,
                tc.tile_pool(name="gps", bufs=2, space="PSUM") as gemmps,
                tc.tile_pool(name="qktp", bufs=2, space="PSUM") as qktps,
                tc.tile_pool(name="sps", bufs=2, space="PSUM") as spool,
                tc.tile_pool(name="avp", bufs=2, space="PSUM") as avps,
            ):
                for t in range(NTILES):
                    xt_t = xpool.tile([128, KT, 128], bf16)
                    nc.sync.dma_start(
                        out=xt_t,
                        in_=xt_d[:, :, t * 128:(t + 1) * 128].rearrange(
                            "k p t -> p k t"
                        ),
                    )
                    qkv_t = qkvpool.tile([128, OC3], bf16)
                    # 8 output chunks of 432 (= 6 heads), 9 k accumulation steps
                    for c in range(8):
                        ps = gemmps.tile([128, 432], f32)
                        for k in range(KT):
                            nc.tensor.matmul(
                                ps,
                                lhsT=xt_t[:, k, :],
                                rhs=wq_sb[:, k, c * 432:(c + 1) * 432],
                                start=(k == 0),
                                stop=(k == KT - 1),
                            )
                        if c % 2 == 0:
                            nc.vector.tensor_copy(
                                out=qkv_t[:, c * 432:(c + 1) * 432], in_=ps
                            )
                        else:
                            nc.scalar.copy(
                                out=qkv_t[:, c * 432:(c + 1) * 432], in_=ps
                            )

                    # RMS stats for q (heads 0..15) and k (16..31); the
                    # 8 zero pad lanes per head do not perturb sum(x^2)
                    sq = sqpool.tile([128, 2 * DIM], bf16)
                    nc.vector.tensor_mul(
                        out=sq, in0=qkv_t[:, 0:2 * DIM], in1=qkv_t[:, 0:2 * DIM]
                    )
                    ssq = stats.tile([128, 32], f32)
                    nc.vector.tensor_reduce(
                        out=ssq,
                        in_=sq.rearrange("p (h d) -> p h d", d=HD),
                        axis=mybir.AxisListType.X,
                        op=ALU.add,
                    )
                    rms = stats.tile([128, 32], f32)
                    nc.scalar.activation(
                        out=rms,
                        in_=ssq,
                        func=ACTF.Sqrt,
                        scale=1.0 / HD,
                        bias=eps_sb[:, 0:1],
                    )
                    rinv = stats.tile([128, 32], f32)
                    nc.vector.reciprocal(out=rinv, in_=rms)
                    qsc = stats.tile([128, 16], f32)
                    nc.vector.tensor_scalar_mul(
                        out=qsc, in0=rinv[:, 0:16], scalar1=SCALE
                    )
                    rowsum = stats.tile([128, 16], f32)
                    rcp = stats.tile([128, 16], f32)

                    if use_wqk:
                        nc.vector.tensor_mul(
                            out=qkv_t[:, DIM:2 * DIM],
                            in0=qkv_t[:, DIM:2 * DIM],
                            in1=wqk_sb,
                        )

                    for g in range(4):
                        p4 = ppool.tile([128, 4, 128], bf16)
                        for j in range(4):
                            h = 4 * g + j
                            qs = qkv_t[:, h * HD:(h + 1) * HD]
                            ks2 = qkv_t[:, DIM + h * HD:DIM + (h + 1) * HD]
                            nc.vector.tensor_scalar_mul(
                                out=ks2, in0=ks2, scalar1=rinv[:, 16 + h:17 + h]
                            )
                            qkt = qktps.tile([HD, 256], bf16)
                            nc.tensor.transpose(qkt[:, 0:128], qs, ident)
                            nc.tensor.transpose(qkt[:, 128:256], ks2, ident)
                            qkts = qktsb.tile([HD, 256], bf16)
                            nc.vector.tensor_copy(out=qkts, in_=qkt)
                            s_ps = spool.tile([128, 128], f32, tag="sp")
                            nc.tensor.matmul(
                                s_ps,
                                lhsT=qkts[:, 0:128],
                                rhs=qkts[:, 128:256],
                                start=True,
                                stop=True,
                            )
                            nc.vector.tensor_add(out=s_ps, in0=s_ps, in1=mask)
                            nc.scalar.activation(
                                out=p4[:, j, :],
                                in_=s_ps,
                                func=ACTF.Exp,
                                scale=qsc[:, h:h + 1],
                                bias=zero_sb[:, 0:1],
                                accum_out=rowsum[:, h:h + 1],
                            )
                        nc.vector.reciprocal(
                            out=rcp[:, 4 * g:4 * g + 4],
                            in_=rowsum[:, 4 * g:4 * g + 4],
                        )
                        # batched 4-head P transpose, HW-validated
                        # [128,N,128] xbar form: [t1, 4h*t2] -> [t2, 4h, t1]
                        pt4 = ptsb.tile([128, 4, 128], bf16)
                        if g % 2 == 0:
                            nc.sync.dma_start_transpose(out=pt4, in_=p4)
                        else:
                            nc.scalar.dma_start_transpose(out=pt4, in_=p4)
                        for j in range(4):
                            h = 4 * g + j
                            vs = qkv_t[:, 2 * DIM + h * HD:2 * DIM + (h + 1) * HD]
                            av = avps.tile([128, HD], f32)
                            nc.tensor.matmul(
                                av, lhsT=pt4[:, j, :], rhs=vs,
                                start=True, stop=True,
                            )
                            nc.scalar.activation(
                                out=o_all[:, t, h * HD:(h + 1) * HD],
                                in_=av,
                                func=ACTF.Copy,
                                scale=rcp[:, h:h + 1],
                            )

            # ---------------- phase 3: transpose O, proj GEMM + bias
            with (
                tc.tile_pool(name="ot", bufs=2) as otpool,
                tc.tile_pool(name="osb", bufs=1) as outpool,
                tc.tile_pool(name="pjp", bufs=4, space="PSUM") as projps,
            ):
                for blk in range(4):
                    ot = otpool.tile([128, KT, 512], bf16)
                    for c in range(KT):
                        for j in range(4):
                            eng = nc.sync if (c + j) % 2 == 0 else nc.scalar
                            eng.dma_start_transpose(
                                out=ot[:, c, j * 128:(j + 1) * 128],
                                in_=o_all[:, blk * 4 + j, c * 128:(c + 1) * 128],
                            )
                    outsb = outpool.tile([128, KT, 512], f32)  # noqa: E501
                    for oc in range(KT):
                        pp = projps.tile([128, 512], f32)
                        for k in range(KT):
                            nc.tensor.matmul(
                                pp,
                                lhsT=wp_sb[:, k, oc * 128:(oc + 1) * 128],
                                rhs=ot[:, k, :],
                                start=(k == 0),
                                stop=(k == KT - 1),
                            )
                        nc.vector.tensor_scalar_add(
                            out=outsb[:, oc, :], in0=pp, scalar1=bp_sb[:, oc:oc + 1]
                        )
                    nc.sync.dma_start(
                        out=out_d[:, :, blk * 512:(blk + 1) * 512].rearrange(
                            "k p t -> p k t"
                        ),
                        in_=outsb,
                    )

    nc.compile()
    return nc


def _get_nc(use_wqk):
    key = ("nc", use_wqk)
    if key not in _CACHE:
        _CACHE[key] = _build_nc(use_wqk)
    return _CACHE[key]


def kernel(x, w_qkv, q_norm_w, k_norm_w, w_proj, b_proj):
    from concourse.bass_utils import run_bass_kernel_spmd

    x = np.asarray(x, dtype=np.float32)
    w_qkv = np.asarray(w_qkv, dtype=np.float32)
    q_norm_w = np.asarray(q_norm_w, dtype=np.float32)
    k_norm_w = np.asarray(k_norm_w, dtype=np.float32)
    w_proj = np.asarray(w_proj, dtype=np.float32)
    b_proj = np.asarray(b_proj, dtype=np.float32)

    B, T, H, W, C = x.shape
    # window partition: [B,T,H,W,C] -> [(n1 n2 n3 B) k1 k2 k3, C] token-major
    xw = x.reshape(B, T // 4, 4, H // 4, 4, W // 4, 4, C)
    xw = xw.transpose(1, 3, 5, 0, 2, 4, 6, 7).reshape(-1, C)  # [16384, 1152]

    wqk = q_norm_w * k_norm_w
    use_wqk = not np.allclose(wqk, 1.0)
    # fold q_norm into the exp scale only when it's uniform ones; general
    # weights are folded into k via the device-side wqk multiply.
    nc = _get_nc(use_wqk)

    wqkv_t = np.ascontiguousarray(w_qkv.T).astype(BF16).reshape(KT, 128, OC3)
    wproj_t = np.ascontiguousarray(w_proj.T).astype(BF16).reshape(KT, 128, DIM)
    bproj = b_proj.reshape(KT, 128).astype(np.float32)

    in_maps = []
    for c in range(NCORES):
        tok = xw[c * TOK_PER_CORE:(c + 1) * TOK_PER_CORE]  # [2048, 1152]
        xt = np.ascontiguousarray(tok.T).astype(BF16).reshape(KT, 128, TOK_PER_CORE)
        m = {"xt": xt, "wqkv": wqkv_t, "wproj": wproj_t, "bproj": bproj}
        if use_wqk:
            m["wqk"] = wqk.astype(BF16).reshape(1, DIM)
        in_maps.append(m)

    res = run_bass_kernel_spmd(nc, in_maps, core_ids=list(range(NCORES)))
    if res.exec_time_ns is not None:
        print(f"HW exec time: {res.exec_time_ns} ns")

    outw = np.empty((16384, C), dtype=np.float32)
    for c in range(NCORES):
        o = res.results[c]["out"].reshape(DIM, TOK_PER_CORE)  # channels-major
        outw[c * TOK_PER_CORE:(c + 1) * TOK_PER_CORE] = o.T

    # window unpartition
    o = outw.reshape(T // 4, H // 4, W // 4, B, 4, 4, 4, C)
    o = o.transpose(3, 0, 4, 1, 5, 2, 6, 7).reshape(B, T, H, W, C)
    return o
